# revision 28
# baseline (speedup 1.0000x reference)
"""MinGRU kernel for Trainium2 (8 NeuronCores, Bass/Tile).

Reference computation (B=4, L=8192, D=512, fp32):
    gates = sigmoid(x @ Wg.T + bg)
    cands = tanh(x @ Wc.T + bc)
    h_t   = (1 - g_t) * h_{t-1} + g_t * c_t   (scan along L, h_0 = 0)

Sharding: core c -> (batch b = c//2, channel half = c%2). Each core computes
its batch's full L range for 256 of the 512 output channels; the scan along L
is per (b, channel) so no cross-core communication is needed.

Layout: all PE inputs are fp16 (half the HBM bytes of fp32; the fp32 x feed
saturated the ~358 GB/s read port). The host packs x per segment as
[128 partitions, NDC*lt] (p-major, contraction-chunk-major inside) so each
segment DMA is ONE contiguous descriptor per partition (8 KB for a 1024-token
segment); weights are packed [128, NDC*EH] the same way. On device, matmuls
keep channels on partitions and tokens on the free axis, which is exactly the
layout tensor_tensor_scan needs (recurrence runs along the free dim).

Engine split per segment (the DVE is the scarce resource — the scan can only
run there, at ~2 cycles/column):
    Act  : a = sigmoid(-z_g - bg) = 1 - g,  c = tanh(z_c + bc)
    DVE  : bn = (a - 1) * c = -g * c  (one scalar_tensor_tensor, both e-tiles)
    DVE  : h = a * h_prev - bn        (tensor_tensor_scan, fp32 state)
Output h is written fp16 ([256, L] per core) and upcast on the host. Dummy
activations at the top pull both 1.3us activation-table loads into the DMA
ramp.

Token segments taper at both ends: a small head segment starts the PE early
(its DMA is tiny), small tail segments shrink the post-last-matmul
act -> pool -> sub -> scan -> h-DMA drain. End-to-end max rel err ~1.5e-3.
"""

import os
import sys

sys.path.insert(0, "/opt/trn_rl_repo")

import numpy as np

import concourse.bacc as bacc
import concourse.bass as bass
import concourse.mybir as mybir
from concourse.bass_utils import run_bass_kernel_spmd
from concourse.tile import TileContext

B, L, D = 4, 8192, 512
NCORES = 8
EH = D // 2          # output channels per core
NET = EH // 128      # e-tiles per core (2)
NDC = D // 128       # contraction chunks (4)
NSUB = 512           # matmul moving free dim (= 1 fp32 PSUM bank)
SEGS = [64, 256, 384, 512, 768, 1024, 1024, 1024, 1024, 1024, 640, 320, 128]
assert sum(SEGS) == L
MAXSEG = max(SEGS)
# a/bn/h tiles reserve column 0 per e-tile as a scan "reset column"
# (see the merged-scan comment below); data lives in columns [1, lt].
M1 = MAXSEG + 1

FP32 = mybir.dt.float32
F16 = mybir.dt.float16
_last_results = None


def build_nc() -> bass.Bass:
    # Bacc (not plain Bass): its compile() runs move_matmul_waits_to_ldweights
    # and generate_event_semaphores, which split multi-sem waits to satisfy the
    # TRN2 per-instruction wait-slot limits walrus enforces.
    nc = bacc.Bacc()

    xS = nc.dram_tensor("xS", [128, NDC * L], F16, kind="ExternalInput")
    # weights split per e-tile so the first matmul is gated by only 384KB
    # (wg half 0 + x seg 0 + wc half 0), not the full 1MB weight set
    wS = [
        [nc.dram_tensor(f"w{p}{e}S", [128, NDC * 128], F16, kind="ExternalInput")
         for e in range(NET)]
        for p in range(2)
    ]
    # biases packed [128, 4]: cols 0..1 = bg per e-tile, 2..3 = bc per e-tile
    bias = nc.dram_tensor("bias", [128, 2 * NET], FP32, kind="ExternalInput")
    h = nc.dram_tensor("h", [EH, L], F16, kind="ExternalOutput")

    op = mybir.AluOpType
    act = mybir.ActivationFunctionType

    with TileContext(nc) as tc:
        with (
            tc.tile_pool(name="consts", bufs=1) as consts,
            tc.tile_pool(name="xpool", bufs=4) as xpool,
            # a/c need deep buffering: their buffers are freed by the DVE
            # scan several segments back, and with too few buffers the Act
            # queue convoys behind the DVE and drains long after the PE.
            tc.tile_pool(name="acpool", bufs=5) as acpool,
            tc.tile_pool(name="work", bufs=3) as work,
            tc.tile_pool(name="hpool", bufs=3) as hpool,
            tc.tile_pool(name="psum", bufs=2, space="PSUM") as psum,
        ):
            # Weight tiles are [128, et, dc, 128] so each per-e-tile DMA
            # lands contiguously (1KB per partition row).
            wg_sb = consts.tile([128, NET, NDC, 128], F16)
            wc_sb = consts.tile([128, NET, NDC, 128], F16)
            x0_sb = xpool.tile([128, NDC * MAXSEG], F16, tag="x", name="x_0")

            # Segment-0's e-tile-0 matmuls need only wg half 0, x seg 0 and
            # wc half 0: those three lead the sync HWDGE queue (it spins up
            # ~1us faster than the scalar queue), then sync stays a pure
            # x-feed. An early-phase x stall is doubly bad: PE sem updates
            # ride later PE instructions, so a stalled matmul also blocks the
            # PREVIOUS segment's activations. The e-tile-1 weight halves
            # (needed ~1us later) ride the scalar queue, triggered BEFORE the
            # table-load dummies so their transfers aren't blocked behind
            # 2.6us of table loads. Biases ride the SWDGE (gpsimd) queue.
            def _wview(sb, et):
                return sb[:, et, :, :].rearrange("p c e -> p (c e)")

            nc.sync.dma_start(_wview(wg_sb, 0), wS[0][0][:])
            nc.sync.dma_start(
                x0_sb[:, : NDC * SEGS[0]], xS[:, : NDC * SEGS[0]]
            )
            nc.sync.dma_start(_wview(wc_sb, 0), wS[1][0][:])
            nc.scalar.dma_start(_wview(wg_sb, 1), wS[0][1][:])
            nc.scalar.dma_start(_wview(wc_sb, 1), wS[1][1][:])

            # Dummy activations with no data dependencies: Bacc places each
            # activation-table load right before the table's first use, so
            # these pull both (sigmoid + tanh) 1.3us table loads into the DMA
            # ramp instead of serializing them in front of the first real
            # activations (which would also stall the PE on PSUM
            # back-pressure).
            scr = consts.tile([128, 2], F16)
            dum = consts.tile([128, 2], F16)
            nc.scalar.memzero(scr)
            nc.scalar.activation(dum, scr, act.Tanh)
            nc.scalar.activation(dum, scr, act.Sigmoid)

            # PE clock warm-up: the tensor engine runs at roughly half clock
            # for its first ~3us of activity (p-state ramp). A burst of dummy
            # matmuls on scratch SBUF during the DMA ramp brings it to full
            # speed before the first real segment, and the <1us gap to the
            # real matmuls is too short for it to re-throttle.
            wdum = consts.tile([128, 128], F16)
            xdum = consts.tile([128, 512], F16)
            nc.vector.memset(wdum, 0.0)
            nc.vector.memset(xdum, 0.0)
            pzw = psum.tile([128, 2, NSUB], FP32, tag="pz0", name="pz_warm")
            for i in range(8):
                nc.tensor.matmul(pzw[:, i % 2, :], wdum, xdum, start=True, stop=True)

            bias_sb = consts.tile([128, 2 * NET], FP32)
            bgn_sb = consts.tile([128, NET], FP32)
            nc.gpsimd.dma_start(bias_sb, bias[:])
            nc.scalar.mul(bgn_sb, bias_sb[:, 0:NET], -1.0)
            bc_sb = bias_sb[:, NET : 2 * NET]

            carry = [None] * NET  # [128, 1] AP of the previous h column

            l0 = 0
            for t, lt in enumerate(SEGS):
                nbk = (lt + NSUB - 1) // NSUB  # PSUM banks this segment
                if t == 0:
                    x_sb = x0_sb
                else:
                    x_sb = xpool.tile([128, NDC * MAXSEG], F16, tag="x", name=f"x_{t}")
                    nc.sync.dma_start(
                        x_sb[:, : NDC * lt], xS[:, NDC * l0 : NDC * (l0 + lt)]
                    )
                a_t = acpool.tile([128, NET, M1], F16, tag="a", name=f"a_{t}")
                c_t = acpool.tile([128, NET, M1], F16, tag="c", name=f"c_{t}")
                bn_t = work.tile([128, NET, M1], F16, tag="b", name=f"b_{t}")
                h_t = hpool.tile([128, NET, M1], F16, tag="h", name=f"h_{t}")
                merged = lt == MAXSEG and carry[0] is not None
                if merged:
                    # Both e-tile scans run as ONE DVE instruction over the
                    # contiguous [et0 | et1] columns. Each e-tile's column 0
                    # is a reset column: a=0 zeroes the scan state (state =
                    # 0*state - bn), and bn = -carry re-injects that e-tile's
                    # carry. a=0 is planted by the idle Pool engine;
                    # bn = (carry*0) - carry is a tiny DVE op that runs right
                    # after the previous scan (same engine, so no cross-engine
                    # semaphore hop on the carry critical path).
                    nc.gpsimd.memset(a_t[:, :, 0:1], 0.0)
                    for et in range(NET):
                        nc.vector.scalar_tensor_tensor(
                            bn_t[:, et, 0:1], carry[et], 0.0, carry[et],
                            op.mult, op.subtract,
                        )
                for et in range(NET):
                    for n in range(nbk):
                        w = min(NSUB, lt - n * NSUB)
                        nsl = slice(1 + n * NSUB, 1 + n * NSUB + w)
                        # One 2-bank PSUM tile per (et, 512-token chunk):
                        # [*, 0, :] = z_g, [*, 1, :] = z_c. bufs=2 per et tag
                        # double-buffers chunks so next-segment matmuls never
                        # convoy behind this segment's activations.
                        pz = psum.tile(
                            [128, 2, NSUB], FP32, tag=f"pz{et}", name=f"pz{et}_{t}_{n}"
                        )
                        for proj in range(2):
                            wsrc = wg_sb if proj == 0 else wc_sb
                            for dc in range(NDC):
                                nc.tensor.matmul(
                                    pz[:, proj, :w],
                                    wsrc[:, et, dc, :],
                                    x_sb[:, dc * lt + n * NSUB : dc * lt + n * NSUB + w],
                                    start=(dc == 0),
                                    stop=(dc == NDC - 1),
                                )
                        # a = sigmoid(-(z_g + bg)) = 1 - g ; c = tanh(z_c + bc)
                        nc.scalar.activation(
                            a_t[:, et, nsl], pz[:, 0, :w], act.Sigmoid,
                            bias=bgn_sb[:, et : et + 1], scale=-1.0,
                        )
                        nc.scalar.activation(
                            c_t[:, et, nsl], pz[:, 1, :w], act.Tanh,
                            bias=bc_sb[:, et : et + 1], scale=1.0,
                        )
                # bneg = (a - 1) * c = -g * c: ONE DVE op covering both
                # e-tiles (3D APs are fine for elementwise ops; only the scan
                # requires 2D).
                nc.vector.scalar_tensor_tensor(
                    bn_t[:, :, 1 : lt + 1], a_t[:, :, 1 : lt + 1], 1.0,
                    c_t[:, :, 1 : lt + 1], op.subtract, op.mult,
                )
                if merged:
                    # h = a * h_prev - bneg (fp32 state in HW, fp16 storage);
                    # reset columns make initial=0 correct for both e-tiles.
                    nc.vector.tensor_tensor_scan(
                        h_t[:, :, :].rearrange("p e l -> p (e l)"),
                        a_t[:, :, :].rearrange("p e l -> p (e l)"),
                        bn_t[:, :, :].rearrange("p e l -> p (e l)"),
                        0.0, op.mult, op.subtract,
                    )
                    for et in range(NET):
                        carry[et] = h_t[:, et, lt : lt + 1]
                else:
                    for et in range(NET):
                        init = 0.0 if carry[et] is None else carry[et]
                        nc.vector.tensor_tensor_scan(
                            h_t[:, et, 1 : lt + 1], a_t[:, et, 1 : lt + 1],
                            bn_t[:, et, 1 : lt + 1], init, op.mult, op.subtract,
                        )
                        carry[et] = h_t[:, et, lt : lt + 1]
                # h writes on the SWDGE queue: keeps the sync HWDGE queue a
                # pure x-feed. One DMA covers both e-tiles.
                nc.gpsimd.dma_start(
                    h[:, l0 : l0 + lt].rearrange("(e p) l -> p e l", p=128),
                    h_t[:, :, 1 : lt + 1],
                )
                l0 += lt
    return nc


def _pack_pmajor(mT: np.ndarray, seg_bounds) -> np.ndarray:
    """[D, N] -> [128, NDC*N] fp16, p-major with contraction-chunk-major
    (then column) inside each segment, so each segment is contiguous per
    partition row."""
    r = mT.reshape(NDC, 128, mT.shape[1])
    parts = [
        np.ascontiguousarray(r[:, :, l0:l1].transpose(1, 0, 2)).reshape(128, -1)
        for l0, l1 in seg_bounds
    ]
    return np.ascontiguousarray(np.concatenate(parts, axis=1).astype(np.float16))


def _in_maps(x, Wg, bg, Wc, bc):
    bounds = []
    l0 = 0
    for lt in SEGS:
        bounds.append((l0, l0 + lt))
        l0 += lt
    maps = []
    xS = {}
    for c in range(NCORES):
        b, eh = c // 2, c % 2
        e0 = eh * EH
        if b not in xS:
            xS[b] = _pack_pmajor(x[b].T.astype(np.float16), bounds)
        bias_pack = np.concatenate(
            [
                bg[e0 : e0 + EH].reshape(NET, 128).T,
                bc[e0 : e0 + EH].reshape(NET, 128).T,
            ],
            axis=1,
        )
        m = {
            "xS": xS[b],
            "bias": np.ascontiguousarray(bias_pack.astype(np.float32)),
        }
        for p, W in ((0, Wg), (1, Wc)):
            for et in range(NET):
                es = e0 + et * 128
                m[f"w{p}{et}S"] = _pack_pmajor(
                    np.ascontiguousarray(W[es : es + 128].T).astype(np.float16),
                    [(0, 128)],
                )
        maps.append(m)
    return maps


def kernel(x, Wg, bg, Wc, bc):
    global _last_results
    x = np.asarray(x, dtype=np.float32)
    Wg = np.asarray(Wg, dtype=np.float32)
    bg = np.asarray(bg, dtype=np.float32)
    Wc = np.asarray(Wc, dtype=np.float32)
    bc = np.asarray(bc, dtype=np.float32)

    nc = build_nc()
    if not nc.is_finalized():
        nc.finalize()
    res = run_bass_kernel_spmd(
        nc,
        _in_maps(x, Wg, bg, Wc, bc),
        list(range(NCORES)),
        tmpdir=os.environ.get("KERNEL_TMPDIR"),
    )
    _last_results = res

    out = np.empty((B, L, D), dtype=np.float32)
    for b in range(B):
        hb = np.concatenate(
            [res.results[2 * b]["h"], res.results[2 * b + 1]["h"]], axis=0
        ).astype(np.float32)
        out[b] = hb.T
    return out


# revision 31
# speedup vs baseline: 1.0084x; 1.0084x over previous
"""MinGRU kernel for Trainium2 (8 NeuronCores, Bass/Tile).

Reference computation (B=4, L=8192, D=512, fp32):
    gates = sigmoid(x @ Wg.T + bg)
    cands = tanh(x @ Wc.T + bc)
    h_t   = (1 - g_t) * h_{t-1} + g_t * c_t   (scan along L, h_0 = 0)

Sharding: core c -> (batch b = c//2, channel half = c%2). Each core computes
its batch's full L range for 256 of the 512 output channels; the scan along L
is per (b, channel) so no cross-core communication is needed.

Layout: all PE inputs are fp16 (half the HBM bytes of fp32; the fp32 x feed
saturated the ~358 GB/s read port). The host packs x per segment as
[128 partitions, NDC*lt] (p-major, contraction-chunk-major inside) so each
segment DMA is ONE contiguous descriptor per partition (8 KB for a 1024-token
segment); weights are packed [128, NDC*EH] the same way. On device, matmuls
keep channels on partitions and tokens on the free axis, which is exactly the
layout tensor_tensor_scan needs (recurrence runs along the free dim).

Engine split per segment (the DVE is the scarce resource — the scan can only
run there, at ~2 cycles/column):
    Act  : a = sigmoid(-z_g - bg) = 1 - g,  c = tanh(z_c + bc)
    DVE  : bn = (a - 1) * c = -g * c  (one scalar_tensor_tensor, both e-tiles)
    DVE  : h = a * h_prev - bn        (tensor_tensor_scan, fp32 state)
For full-size segments both e-tile scans run as ONE instruction over the
contiguous [et0 | et1] columns, using per-e-tile "reset columns" (a=0 zeroes
the fp32 scan state and bn=-carry re-injects the carry) so the hand-off is
free. Output h is written fp16 ([256, L] per core) and upcast on the host.

Startup hiding: dummy activations pull both 1.3us activation-table loads into
the DMA ramp, and a burst of dummy matmuls warms the PE out of its half-clock
p-state before the first real segment. Token segments taper at both ends: a
small head segment starts the PE early (its DMA is tiny and the e-tile-0
weight halves lead the queue), small tail segments shrink the post-last-matmul
act -> bneg -> scan -> h-DMA drain. End-to-end max rel err ~1.4e-3.
"""

import os
import sys

sys.path.insert(0, "/opt/trn_rl_repo")

import numpy as np

import concourse.bacc as bacc
import concourse.bass as bass
import concourse.mybir as mybir
from concourse.bass_utils import run_bass_kernel_spmd
from concourse.tile import TileContext

B, L, D = 4, 8192, 512
NCORES = 8
EH = D // 2          # output channels per core
NET = EH // 128      # e-tiles per core (2)
NDC = D // 128       # contraction chunks (4)
NSUB = 512           # matmul moving free dim (= 1 fp32 PSUM bank)
SEGS = [128, 256, 384, 512, 768, 1024, 1024, 1024, 1024, 1024, 640, 256, 128]
assert sum(SEGS) == L
MAXSEG = max(SEGS)
# a/bn/h tiles reserve column 0 per e-tile as a scan "reset column"
# (see the merged-scan comment below); data lives in columns [1, lt].
M1 = MAXSEG + 1

FP32 = mybir.dt.float32
F16 = mybir.dt.float16
_last_results = None


def build_nc() -> bass.Bass:
    # Bacc (not plain Bass): its compile() runs move_matmul_waits_to_ldweights
    # and generate_event_semaphores, which split multi-sem waits to satisfy the
    # TRN2 per-instruction wait-slot limits walrus enforces.
    nc = bacc.Bacc()

    xS = nc.dram_tensor("xS", [128, NDC * L], F16, kind="ExternalInput")
    # weights split per e-tile so the first matmul is gated by only 384KB
    # (wg half 0 + x seg 0 + wc half 0), not the full 1MB weight set
    wS = [
        [nc.dram_tensor(f"w{p}{e}S", [128, NDC * 128], F16, kind="ExternalInput")
         for e in range(NET)]
        for p in range(2)
    ]
    # biases packed [128, 4]: cols 0..1 = bg per e-tile, 2..3 = bc per e-tile
    bias = nc.dram_tensor("bias", [128, 2 * NET], FP32, kind="ExternalInput")
    h = nc.dram_tensor("h", [EH, L], F16, kind="ExternalOutput")

    op = mybir.AluOpType
    act = mybir.ActivationFunctionType

    with TileContext(nc) as tc:
        with (
            tc.tile_pool(name="consts", bufs=1) as consts,
            tc.tile_pool(name="xpool", bufs=4) as xpool,
            # a/c need deep buffering: their buffers are freed by the DVE
            # scan several segments back, and with too few buffers the Act
            # queue convoys behind the DVE and drains long after the PE.
            tc.tile_pool(name="acpool", bufs=5) as acpool,
            tc.tile_pool(name="work", bufs=3) as work,
            tc.tile_pool(name="hpool", bufs=3) as hpool,
            tc.tile_pool(name="psum", bufs=2, space="PSUM") as psum,
        ):
            # Weight tiles are [128, et, dc, 128] so each per-e-tile DMA
            # lands contiguously (1KB per partition row).
            wg_sb = consts.tile([128, NET, NDC, 128], F16)
            wc_sb = consts.tile([128, NET, NDC, 128], F16)
            x0_sb = xpool.tile([128, NDC * MAXSEG], F16, tag="x", name="x_0")

            # Segment-0's e-tile-0 matmuls need only wg half 0, x seg 0 and
            # wc half 0: those three lead the sync HWDGE queue (it spins up
            # ~1us faster than the scalar queue), then sync stays a pure
            # x-feed. An early-phase x stall is doubly bad: PE sem updates
            # ride later PE instructions, so a stalled matmul also blocks the
            # PREVIOUS segment's activations. The e-tile-1 weight halves
            # (needed ~1us later) ride the scalar queue, triggered BEFORE the
            # table-load dummies so their transfers aren't blocked behind
            # 2.6us of table loads. Biases ride the SWDGE (gpsimd) queue.
            def _wview(sb, et):
                return sb[:, et, :, :].rearrange("p c e -> p (c e)")

            nc.sync.dma_start(_wview(wg_sb, 0), wS[0][0][:])
            nc.sync.dma_start(
                x0_sb[:, : NDC * SEGS[0]], xS[:, : NDC * SEGS[0]]
            )
            nc.sync.dma_start(_wview(wc_sb, 0), wS[1][0][:])
            nc.scalar.dma_start(_wview(wg_sb, 1), wS[0][1][:])
            nc.scalar.dma_start(_wview(wc_sb, 1), wS[1][1][:])

            # Dummy activations with no data dependencies: Bacc places each
            # activation-table load right before the table's first use, so
            # these pull both (sigmoid + tanh) 1.3us table loads into the DMA
            # ramp instead of serializing them in front of the first real
            # activations (which would also stall the PE on PSUM
            # back-pressure).
            scr = consts.tile([128, 2], F16)
            dum = consts.tile([128, 2], F16)
            nc.scalar.memzero(scr)
            nc.scalar.activation(dum, scr, act.Tanh)
            nc.scalar.activation(dum, scr, act.Sigmoid)

            # PE clock warm-up: the tensor engine runs at roughly half clock
            # for its first ~3us of activity (p-state ramp). A burst of dummy
            # matmuls on scratch SBUF during the DMA ramp brings it to full
            # speed before the first real segment, and the <1us gap to the
            # real matmuls is too short for it to re-throttle.
            wdum = consts.tile([128, 128], F16)
            xdum = consts.tile([128, 512], F16)
            nc.vector.memset(wdum, 0.0)
            nc.vector.memset(xdum, 0.0)
            pzw = psum.tile([128, 2, NSUB], FP32, tag="pz0", name="pz_warm")
            for i in range(8):
                nc.tensor.matmul(pzw[:, i % 2, :], wdum, xdum, start=True, stop=True)

            bias_sb = consts.tile([128, 2 * NET], FP32)
            bgn_sb = consts.tile([128, NET], FP32)
            nc.gpsimd.dma_start(bias_sb, bias[:])
            nc.scalar.mul(bgn_sb, bias_sb[:, 0:NET], -1.0)
            bc_sb = bias_sb[:, NET : 2 * NET]

            carry = [None] * NET  # [128, 1] AP of the previous h column

            l0 = 0
            for t, lt in enumerate(SEGS):
                nbk = (lt + NSUB - 1) // NSUB  # PSUM banks this segment
                if t == 0:
                    x_sb = x0_sb
                else:
                    x_sb = xpool.tile([128, NDC * MAXSEG], F16, tag="x", name=f"x_{t}")
                    nc.sync.dma_start(
                        x_sb[:, : NDC * lt], xS[:, NDC * l0 : NDC * (l0 + lt)]
                    )
                a_t = acpool.tile([128, NET, M1], F16, tag="a", name=f"a_{t}")
                c_t = acpool.tile([128, NET, M1], F16, tag="c", name=f"c_{t}")
                bn_t = work.tile([128, NET, M1], F16, tag="b", name=f"b_{t}")
                h_t = hpool.tile([128, NET, M1], F16, tag="h", name=f"h_{t}")
                merged = lt == MAXSEG and carry[0] is not None
                if merged:
                    # Both e-tile scans run as ONE DVE instruction over the
                    # contiguous [et0 | et1] columns. Each e-tile's column 0
                    # is a reset column: a=0 zeroes the scan state (state =
                    # 0*state - bn), and bn = -carry re-injects that e-tile's
                    # carry — so the carry hand-off costs no extra DVE work.
                    # The a=0 / bn=-carry columns are planted by the
                    # otherwise-idle Pool engine.
                    nc.gpsimd.memset(a_t[:, :, 0:1], 0.0)
                    for et in range(NET):
                        nc.gpsimd.tensor_tensor(
                            bn_t[:, et, 0:1], scr[:, 0:1], carry[et], op.subtract
                        )
                for et in range(NET):
                    for n in range(nbk):
                        w = min(NSUB, lt - n * NSUB)
                        nsl = slice(1 + n * NSUB, 1 + n * NSUB + w)
                        # One 2-bank PSUM tile per (et, 512-token chunk):
                        # [*, 0, :] = z_g, [*, 1, :] = z_c. bufs=2 per et tag
                        # double-buffers chunks so next-segment matmuls never
                        # convoy behind this segment's activations.
                        pz = psum.tile(
                            [128, 2, NSUB], FP32, tag=f"pz{et}", name=f"pz{et}_{t}_{n}"
                        )
                        for proj in range(2):
                            wsrc = wg_sb if proj == 0 else wc_sb
                            for dc in range(NDC):
                                nc.tensor.matmul(
                                    pz[:, proj, :w],
                                    wsrc[:, et, dc, :],
                                    x_sb[:, dc * lt + n * NSUB : dc * lt + n * NSUB + w],
                                    start=(dc == 0),
                                    stop=(dc == NDC - 1),
                                )
                        # a = sigmoid(-(z_g + bg)) = 1 - g ; c = tanh(z_c + bc)
                        nc.scalar.activation(
                            a_t[:, et, nsl], pz[:, 0, :w], act.Sigmoid,
                            bias=bgn_sb[:, et : et + 1], scale=-1.0,
                        )
                        nc.scalar.activation(
                            c_t[:, et, nsl], pz[:, 1, :w], act.Tanh,
                            bias=bc_sb[:, et : et + 1], scale=1.0,
                        )
                # bneg = (a - 1) * c = -g * c: ONE DVE op covering both
                # e-tiles (3D APs are fine for elementwise ops; only the scan
                # requires 2D).
                nc.vector.scalar_tensor_tensor(
                    bn_t[:, :, 1 : lt + 1], a_t[:, :, 1 : lt + 1], 1.0,
                    c_t[:, :, 1 : lt + 1], op.subtract, op.mult,
                )
                if merged:
                    # h = a * h_prev - bneg (fp32 state in HW, fp16 storage);
                    # reset columns make initial=0 correct for both e-tiles.
                    nc.vector.tensor_tensor_scan(
                        h_t[:, :, :].rearrange("p e l -> p (e l)"),
                        a_t[:, :, :].rearrange("p e l -> p (e l)"),
                        bn_t[:, :, :].rearrange("p e l -> p (e l)"),
                        0.0, op.mult, op.subtract,
                    )
                    for et in range(NET):
                        carry[et] = h_t[:, et, lt : lt + 1]
                else:
                    for et in range(NET):
                        init = 0.0 if carry[et] is None else carry[et]
                        nc.vector.tensor_tensor_scan(
                            h_t[:, et, 1 : lt + 1], a_t[:, et, 1 : lt + 1],
                            bn_t[:, et, 1 : lt + 1], init, op.mult, op.subtract,
                        )
                        carry[et] = h_t[:, et, lt : lt + 1]
                # h writes on the SWDGE queue: keeps the sync HWDGE queue a
                # pure x-feed. One DMA covers both e-tiles.
                nc.gpsimd.dma_start(
                    h[:, l0 : l0 + lt].rearrange("(e p) l -> p e l", p=128),
                    h_t[:, :, 1 : lt + 1],
                )
                l0 += lt
    return nc


def _pack_pmajor(mT: np.ndarray, seg_bounds) -> np.ndarray:
    """[D, N] -> [128, NDC*N] fp16, p-major with contraction-chunk-major
    (then column) inside each segment, so each segment is contiguous per
    partition row."""
    r = mT.reshape(NDC, 128, mT.shape[1])
    parts = [
        np.ascontiguousarray(r[:, :, l0:l1].transpose(1, 0, 2)).reshape(128, -1)
        for l0, l1 in seg_bounds
    ]
    return np.ascontiguousarray(np.concatenate(parts, axis=1).astype(np.float16))


def _in_maps(x, Wg, bg, Wc, bc):
    bounds = []
    l0 = 0
    for lt in SEGS:
        bounds.append((l0, l0 + lt))
        l0 += lt
    maps = []
    xS = {}
    for c in range(NCORES):
        b, eh = c // 2, c % 2
        e0 = eh * EH
        if b not in xS:
            xS[b] = _pack_pmajor(x[b].T.astype(np.float16), bounds)
        bias_pack = np.concatenate(
            [
                bg[e0 : e0 + EH].reshape(NET, 128).T,
                bc[e0 : e0 + EH].reshape(NET, 128).T,
            ],
            axis=1,
        )
        m = {
            "xS": xS[b],
            "bias": np.ascontiguousarray(bias_pack.astype(np.float32)),
        }
        for p, W in ((0, Wg), (1, Wc)):
            for et in range(NET):
                es = e0 + et * 128
                m[f"w{p}{et}S"] = _pack_pmajor(
                    np.ascontiguousarray(W[es : es + 128].T).astype(np.float16),
                    [(0, 128)],
                )
        maps.append(m)
    return maps


def kernel(x, Wg, bg, Wc, bc):
    global _last_results
    x = np.asarray(x, dtype=np.float32)
    Wg = np.asarray(Wg, dtype=np.float32)
    bg = np.asarray(bg, dtype=np.float32)
    Wc = np.asarray(Wc, dtype=np.float32)
    bc = np.asarray(bc, dtype=np.float32)

    nc = build_nc()
    if not nc.is_finalized():
        nc.finalize()
    res = run_bass_kernel_spmd(
        nc,
        _in_maps(x, Wg, bg, Wc, bc),
        list(range(NCORES)),
        tmpdir=os.environ.get("KERNEL_TMPDIR"),
    )
    _last_results = res

    out = np.empty((B, L, D), dtype=np.float32)
    for b in range(B):
        hb = np.concatenate(
            [res.results[2 * b]["h"], res.results[2 * b + 1]["h"]], axis=0
        ).astype(np.float32)
        out[b] = hb.T
    return out


# revision 33
# speedup vs baseline: 1.0096x; 1.0013x over previous
"""MinGRU kernel for Trainium2 (8 NeuronCores, Bass/Tile).

Reference computation (B=4, L=8192, D=512, fp32):
    gates = sigmoid(x @ Wg.T + bg)
    cands = tanh(x @ Wc.T + bc)
    h_t   = (1 - g_t) * h_{t-1} + g_t * c_t   (scan along L, h_0 = 0)

Sharding: core c -> (batch b = c//2, channel half = c%2). Each core computes
its batch's full L range for 256 of the 512 output channels; the scan along L
is per (b, channel) so no cross-core communication is needed.

Layout: all PE inputs are fp16 (half the HBM bytes of fp32; the fp32 x feed
saturated the ~358 GB/s read port). The host packs x per segment as
[128 partitions, NDC*lt] (p-major, contraction-chunk-major inside) so each
segment DMA is ONE contiguous descriptor per partition (8 KB for a 1024-token
segment); weights are packed [128, NDC*EH] the same way. On device, matmuls
keep channels on partitions and tokens on the free axis, which is exactly the
layout tensor_tensor_scan needs (recurrence runs along the free dim).

Engine split per segment (the DVE is the scarce resource — the scan can only
run there, at ~2 cycles/column):
    Act  : a = sigmoid(-z_g - bg) = 1 - g,  c = tanh(z_c + bc)
    DVE  : bn = (a - 1) * c = -g * c  (one scalar_tensor_tensor, both e-tiles)
    DVE  : h = a * h_prev - bn        (tensor_tensor_scan, fp32 state)
For full-size segments both e-tile scans run as ONE instruction over the
contiguous [et0 | et1] columns, using per-e-tile "reset columns" (a=0 zeroes
the fp32 scan state and bn=-carry re-injects the carry) so the hand-off is
free. Output h is written fp16 ([256, L] per core) and upcast on the host.

Startup hiding: dummy activations pull both 1.3us activation-table loads into
the DMA ramp, and a burst of dummy matmuls warms the PE out of its half-clock
p-state before the first real segment. Token segments taper at both ends: a
small head segment starts the PE early (its DMA is tiny and the e-tile-0
weight halves lead the queue), small tail segments shrink the post-last-matmul
act -> bneg -> scan -> h-DMA drain. End-to-end max rel err ~1.4e-3.
"""

import os
import sys

sys.path.insert(0, "/opt/trn_rl_repo")

import numpy as np

import concourse.bacc as bacc
import concourse.bass as bass
import concourse.mybir as mybir
from concourse.bass_utils import run_bass_kernel_spmd
from concourse.tile import TileContext

B, L, D = 4, 8192, 512
NCORES = 8
EH = D // 2          # output channels per core
NET = EH // 128      # e-tiles per core (2)
NDC = D // 128       # contraction chunks (4)
NSUB = 512           # matmul moving free dim (= 1 fp32 PSUM bank)
SEGS = [128, 256, 384, 512, 768, 1024, 1024, 1024, 1024, 1024, 1024]
assert sum(SEGS) == L
MAXSEG = max(SEGS)
# a/bn/h tiles reserve column 0 per e-tile as a scan "reset column"
# (see the merged-scan comment below); data lives in columns [1, lt].
M1 = MAXSEG + 1

FP32 = mybir.dt.float32
F16 = mybir.dt.float16
_last_results = None


def build_nc() -> bass.Bass:
    # Bacc (not plain Bass): its compile() runs move_matmul_waits_to_ldweights
    # and generate_event_semaphores, which split multi-sem waits to satisfy the
    # TRN2 per-instruction wait-slot limits walrus enforces.
    nc = bacc.Bacc()

    xS = nc.dram_tensor("xS", [128, NDC * L], F16, kind="ExternalInput")
    # weights split per e-tile so the first matmul is gated by only 384KB
    # (wg half 0 + x seg 0 + wc half 0), not the full 1MB weight set
    wS = [
        [nc.dram_tensor(f"w{p}{e}S", [128, NDC * 128], F16, kind="ExternalInput")
         for e in range(NET)]
        for p in range(2)
    ]
    # biases packed [128, 4]: cols 0..1 = bg per e-tile, 2..3 = bc per e-tile
    bias = nc.dram_tensor("bias", [128, 2 * NET], FP32, kind="ExternalInput")
    h = nc.dram_tensor("h", [EH, L], F16, kind="ExternalOutput")

    op = mybir.AluOpType
    act = mybir.ActivationFunctionType

    with TileContext(nc) as tc:
        with (
            tc.tile_pool(name="consts", bufs=1) as consts,
            tc.tile_pool(name="xpool", bufs=4) as xpool,
            # a/c need deep buffering: their buffers are freed by the DVE
            # scan several segments back, and with too few buffers the Act
            # queue convoys behind the DVE and drains long after the PE.
            tc.tile_pool(name="acpool", bufs=5) as acpool,
            tc.tile_pool(name="work", bufs=3) as work,
            tc.tile_pool(name="hpool", bufs=3) as hpool,
            tc.tile_pool(name="psum", bufs=2, space="PSUM") as psum,
        ):
            # Weight tiles are [128, et, dc, 128] so each per-e-tile DMA
            # lands contiguously (1KB per partition row).
            wg_sb = consts.tile([128, NET, NDC, 128], F16)
            wc_sb = consts.tile([128, NET, NDC, 128], F16)
            x0_sb = xpool.tile([128, NDC * MAXSEG], F16, tag="x", name="x_0")

            # Segment-0's e-tile-0 matmuls need only wg half 0, x seg 0 and
            # wc half 0: those three lead the sync HWDGE queue (it spins up
            # ~1us faster than the scalar queue), then sync stays a pure
            # x-feed. An early-phase x stall is doubly bad: PE sem updates
            # ride later PE instructions, so a stalled matmul also blocks the
            # PREVIOUS segment's activations. The e-tile-1 weight halves
            # (needed ~1us later) ride the scalar queue, triggered BEFORE the
            # table-load dummies so their transfers aren't blocked behind
            # 2.6us of table loads. Biases ride the SWDGE (gpsimd) queue.
            def _wview(sb, et):
                return sb[:, et, :, :].rearrange("p c e -> p (c e)")

            nc.sync.dma_start(_wview(wg_sb, 0), wS[0][0][:])
            nc.sync.dma_start(
                x0_sb[:, : NDC * SEGS[0]], xS[:, : NDC * SEGS[0]]
            )
            nc.sync.dma_start(_wview(wc_sb, 0), wS[1][0][:])
            nc.scalar.dma_start(_wview(wg_sb, 1), wS[0][1][:])
            nc.scalar.dma_start(_wview(wc_sb, 1), wS[1][1][:])

            # Dummy activations with no data dependencies: Bacc places each
            # activation-table load right before the table's first use, so
            # these pull both (sigmoid + tanh) 1.3us table loads into the DMA
            # ramp instead of serializing them in front of the first real
            # activations (which would also stall the PE on PSUM
            # back-pressure).
            scr = consts.tile([128, 2], F16)
            dum = consts.tile([128, 2], F16)
            nc.scalar.memzero(scr)
            nc.scalar.activation(dum, scr, act.Tanh)
            nc.scalar.activation(dum, scr, act.Sigmoid)

            # PE clock warm-up: the tensor engine runs at roughly half clock
            # for its first ~3us of activity (p-state ramp). A burst of dummy
            # matmuls on scratch SBUF during the DMA ramp brings it to full
            # speed before the first real segment, and the <1us gap to the
            # real matmuls is too short for it to re-throttle.
            wdum = consts.tile([128, 128], F16)
            xdum = consts.tile([128, 512], F16)
            nc.vector.memset(wdum, 0.0)
            nc.vector.memset(xdum, 0.0)
            pzw = psum.tile([128, 2, NSUB], FP32, tag="pz0", name="pz_warm")
            for i in range(8):
                nc.tensor.matmul(pzw[:, i % 2, :], wdum, xdum, start=True, stop=True)

            bias_sb = consts.tile([128, 2 * NET], FP32)
            bgn_sb = consts.tile([128, NET], FP32)
            nc.gpsimd.dma_start(bias_sb, bias[:])
            nc.scalar.mul(bgn_sb, bias_sb[:, 0:NET], -1.0)
            bc_sb = bias_sb[:, NET : 2 * NET]

            carry = [None] * NET  # [128, 1] AP of the previous h column

            l0 = 0
            for t, lt in enumerate(SEGS):
                nbk = (lt + NSUB - 1) // NSUB  # PSUM banks this segment
                if t == 0:
                    x_sb = x0_sb
                else:
                    x_sb = xpool.tile([128, NDC * MAXSEG], F16, tag="x", name=f"x_{t}")
                    nc.sync.dma_start(
                        x_sb[:, : NDC * lt], xS[:, NDC * l0 : NDC * (l0 + lt)]
                    )
                a_t = acpool.tile([128, NET, M1], F16, tag="a", name=f"a_{t}")
                c_t = acpool.tile([128, NET, M1], F16, tag="c", name=f"c_{t}")
                bn_t = work.tile([128, NET, M1], F16, tag="b", name=f"b_{t}")
                h_t = hpool.tile([128, NET, M1], F16, tag="h", name=f"h_{t}")
                merged = lt == MAXSEG and carry[0] is not None
                if merged:
                    # Both e-tile scans run as ONE DVE instruction over the
                    # contiguous [et0 | et1] columns. Each e-tile's column 0
                    # is a reset column: a=0 zeroes the scan state (state =
                    # 0*state - bn), and bn = -carry re-injects that e-tile's
                    # carry — so the carry hand-off costs no extra DVE work.
                    # The a=0 / bn=-carry columns are planted by the
                    # otherwise-idle Pool engine.
                    nc.gpsimd.memset(a_t[:, :, 0:1], 0.0)
                    for et in range(NET):
                        nc.gpsimd.tensor_tensor(
                            bn_t[:, et, 0:1], scr[:, 0:1], carry[et], op.subtract
                        )
                for et in range(NET):
                    for n in range(nbk):
                        w = min(NSUB, lt - n * NSUB)
                        nsl = slice(1 + n * NSUB, 1 + n * NSUB + w)
                        # One 2-bank PSUM tile per (et, 512-token chunk):
                        # [*, 0, :] = z_g, [*, 1, :] = z_c. bufs=2 per et tag
                        # double-buffers chunks so next-segment matmuls never
                        # convoy behind this segment's activations.
                        pz = psum.tile(
                            [128, 2, NSUB], FP32, tag=f"pz{et}", name=f"pz{et}_{t}_{n}"
                        )
                        for proj in range(2):
                            wsrc = wg_sb if proj == 0 else wc_sb
                            for dc in range(NDC):
                                nc.tensor.matmul(
                                    pz[:, proj, :w],
                                    wsrc[:, et, dc, :],
                                    x_sb[:, dc * lt + n * NSUB : dc * lt + n * NSUB + w],
                                    start=(dc == 0),
                                    stop=(dc == NDC - 1),
                                )
                        # a = sigmoid(-(z_g + bg)) = 1 - g ; c = tanh(z_c + bc)
                        nc.scalar.activation(
                            a_t[:, et, nsl], pz[:, 0, :w], act.Sigmoid,
                            bias=bgn_sb[:, et : et + 1], scale=-1.0,
                        )
                        nc.scalar.activation(
                            c_t[:, et, nsl], pz[:, 1, :w], act.Tanh,
                            bias=bc_sb[:, et : et + 1], scale=1.0,
                        )
                # bneg = (a - 1) * c = -g * c: ONE DVE op covering both
                # e-tiles (3D APs are fine for elementwise ops; only the scan
                # requires 2D).
                nc.vector.scalar_tensor_tensor(
                    bn_t[:, :, 1 : lt + 1], a_t[:, :, 1 : lt + 1], 1.0,
                    c_t[:, :, 1 : lt + 1], op.subtract, op.mult,
                )
                if merged:
                    # h = a * h_prev - bneg (fp32 state in HW, fp16 storage);
                    # reset columns make initial=0 correct for both e-tiles.
                    nc.vector.tensor_tensor_scan(
                        h_t[:, :, :].rearrange("p e l -> p (e l)"),
                        a_t[:, :, :].rearrange("p e l -> p (e l)"),
                        bn_t[:, :, :].rearrange("p e l -> p (e l)"),
                        0.0, op.mult, op.subtract,
                    )
                    for et in range(NET):
                        carry[et] = h_t[:, et, lt : lt + 1]
                else:
                    for et in range(NET):
                        init = 0.0 if carry[et] is None else carry[et]
                        nc.vector.tensor_tensor_scan(
                            h_t[:, et, 1 : lt + 1], a_t[:, et, 1 : lt + 1],
                            bn_t[:, et, 1 : lt + 1], init, op.mult, op.subtract,
                        )
                        carry[et] = h_t[:, et, lt : lt + 1]
                # h writes on the SWDGE queue: keeps the sync HWDGE queue a
                # pure x-feed. One DMA covers both e-tiles.
                nc.gpsimd.dma_start(
                    h[:, l0 : l0 + lt].rearrange("(e p) l -> p e l", p=128),
                    h_t[:, :, 1 : lt + 1],
                )
                l0 += lt
    return nc


def _pack_pmajor(mT: np.ndarray, seg_bounds) -> np.ndarray:
    """[D, N] -> [128, NDC*N] fp16, p-major with contraction-chunk-major
    (then column) inside each segment, so each segment is contiguous per
    partition row."""
    r = mT.reshape(NDC, 128, mT.shape[1])
    parts = [
        np.ascontiguousarray(r[:, :, l0:l1].transpose(1, 0, 2)).reshape(128, -1)
        for l0, l1 in seg_bounds
    ]
    return np.ascontiguousarray(np.concatenate(parts, axis=1).astype(np.float16))


def _in_maps(x, Wg, bg, Wc, bc):
    bounds = []
    l0 = 0
    for lt in SEGS:
        bounds.append((l0, l0 + lt))
        l0 += lt
    maps = []
    xS = {}
    for c in range(NCORES):
        b, eh = c // 2, c % 2
        e0 = eh * EH
        if b not in xS:
            xS[b] = _pack_pmajor(x[b].T.astype(np.float16), bounds)
        bias_pack = np.concatenate(
            [
                bg[e0 : e0 + EH].reshape(NET, 128).T,
                bc[e0 : e0 + EH].reshape(NET, 128).T,
            ],
            axis=1,
        )
        m = {
            "xS": xS[b],
            "bias": np.ascontiguousarray(bias_pack.astype(np.float32)),
        }
        for p, W in ((0, Wg), (1, Wc)):
            for et in range(NET):
                es = e0 + et * 128
                m[f"w{p}{et}S"] = _pack_pmajor(
                    np.ascontiguousarray(W[es : es + 128].T).astype(np.float16),
                    [(0, 128)],
                )
        maps.append(m)
    return maps


def kernel(x, Wg, bg, Wc, bc):
    global _last_results
    x = np.asarray(x, dtype=np.float32)
    Wg = np.asarray(Wg, dtype=np.float32)
    bg = np.asarray(bg, dtype=np.float32)
    Wc = np.asarray(Wc, dtype=np.float32)
    bc = np.asarray(bc, dtype=np.float32)

    nc = build_nc()
    if not nc.is_finalized():
        nc.finalize()
    res = run_bass_kernel_spmd(
        nc,
        _in_maps(x, Wg, bg, Wc, bc),
        list(range(NCORES)),
        tmpdir=os.environ.get("KERNEL_TMPDIR"),
    )
    _last_results = res

    out = np.empty((B, L, D), dtype=np.float32)
    for b in range(B):
        hb = np.concatenate(
            [res.results[2 * b]["h"], res.results[2 * b + 1]["h"]], axis=0
        ).astype(np.float32)
        out[b] = hb.T
    return out
